# revision 31
# baseline (speedup 1.0000x reference)
"""Trainium2 Bass kernel for nn_AttentionLayer (B=4, N=4096, D=128).

Computation (per reference):
    Q = h @ Wq + bq ; K = h @ Wk + bk ; V = h @ Wv + bv          [B, N, 128]
    scores = einsum("bnd,bmd->bnm", K, Q) / sqrt(128)            [B, N, N]
    attn = softmax(scores, axis=-1)
    out = einsum("bnm,bmd->bnd", attn, V)                        [B, N, N->128]

Sharding: 8 cores = 4 batches x 2 chunks of 2048 K-rows (output rows).
Fully data-parallel SPMD - no collectives. Each core receives its batch's
h rows PERMUTED so that its own K-chunk rows come first: softmax/PV reduce
over the m (Q/V) index, which is order-independent, so the permutation only
fixes which rows the core treats as its K chunk (the first 2048).

Design (v4):
  - ACT exp of the full [2048, 4096] score block (64 x [128,1024] tiles)
    is the hard per-core floor (~66us at 1.2GHz x 128 lanes); everything
    else is organized to keep ACT saturated.
  - All matmul operands fp16 (rel err vs fp32 reference ~3e-4, gate 2e-2).
  - Main loop per (window nh, q-tile mi): PE scores -> ACT exp -> PE PV
    accumulate. The softmax denominator is NOT a third PE pass over the
    score block: DVE pairwise-adds the 32 fp16 e-tiles of each window
    (31 adds at the 2x packed mode) and a single ones[128,128] matmul on
    the reduced esum gives the column sums broadcast over partitions
    (2 x 1024 cols of PE instead of 64 x 1024).
  - PE transposes (h^T, V-natural, output) with a bf16/fp16 identity;
    DMA-XBAR transposes and gpsimd bulk ops measured far slower on HW.
  - Window finalize (den matmul, recip, normalize, transpose, store) is
    deferred into the next window so PE/ACT never wait on the den
    round-trip; the PV accumulator is drained to SBUF at window end so
    the single PSUM o-slot frees immediately.
  - Ping-pong software pipeline across repeat iterations: the For_i body
    holds TWO passes; each pass interleaves the NEXT pass's h-transposes
    and Q/K/V projections (into the other buffer of each double-buffered
    operand tile) into its own slack, so the steady-state per-pass cost
    has no projection prologue.
"""

import math
from contextlib import ExitStack

import numpy as np

import concourse.bass as bass
import concourse.mybir as mybir
import concourse.tile as tile
from concourse.bass_utils import run_bass_kernel_spmd
from concourse.masks import make_identity
from concourse.tile import ScopedClock

F32 = mybir.dt.float32
F16 = mybir.dt.float16

B, N, D = 4, 4096, 128
NCORES = 8
CHUNK = N * B // NCORES  # 2048 output rows per core
NW = 1024  # n processed per PSUM-resident accumulation group
SCALE = 1.0 / math.sqrt(D)


def _patched_drain_and_barrier(self, tick_clock, wait_clock):
    # This walrus build rejects multiple sync waits on the Drain CTRL
    # instruction. Carry the waits on preceding SP nops (same engine =>
    # program order) and leave the drain nearly bare.
    nc = self.nc
    carrier = nc.sync.nop(nofuse=True, hint="drain_waits")
    wait_clock.add_sem_waits(carrier.ins, ScopedClock({None: tick_clock.global_clock}))
    si = carrier.ins.sync_info
    waits = list(si.on_wait) if si is not None else []
    if len(waits) > 1:
        by_name = {}
        for h in self.sems.allocated().values():
            by_name[getattr(h, "name", None) or str(h)] = h
        si.on_wait = [waits[0]]
        for w in waits[1:]:
            n = nc.sync.nop(nofuse=True, hint="drain_waits2")
            n.wait_op(by_name[w.ant_name], w.wait_value, "sem-ge")
    nc.sync.drain()
    nc.all_engine_barrier()
    assert self.sems is not None
    popped = nc._tile_sem_poison_stack.pop()
    assert popped is self._sem_poison
    nc.clear_and_free_semaphores(list(self.sems.allocated().values()))
    nc.all_engine_barrier()


def ts(i, sz):
    return slice(i * sz, (i + 1) * sz)


def _split_excess_waits(nc, maxw=1):
    # This walrus build allows at most ~1 sync wait per lowered instruction.
    # Hoist excess waits onto preceding same-engine NoOps.
    cnt = 0
    for f in nc.m.functions:
        for bb in f.blocks:
            out = []
            for inst in bb.instructions:
                si = inst.sync_info
                waits = list(si.on_wait) if si is not None else []
                if len(waits) > maxw:
                    for w in waits[: len(waits) - maxw]:
                        nop = mybir.InstNoOp(
                            name=f"{inst.name}-hw{cnt}",
                            engine=inst.engine,
                            ins=[],
                            outs=[],
                            sync_info=mybir.SyncInfo(on_wait=[w], on_update=[]),
                        )
                        out.append(nop)
                        cnt += 1
                    si.on_wait = waits[len(waits) - maxw :]
                out.append(inst)
            bb.instructions = out
    return cnt


def build_nc(n=N, chunk=CHUNK, nw=NW, split_waits=True, repeat=1,
             pingpong=False, **_flags):
    M_TILES = n // 128  # 32
    NH = chunk // nw  # 2
    MMW = min(512, nw)  # matmul moving width (PSUM bank cap for fp32)
    GRP = 8  # tiles per prep group (1024 cols)
    N_GRP = M_TILES // GRP  # 4
    GW = GRP * 128
    TOT = NH * M_TILES
    tile.TileContext._drain_and_barrier = _patched_drain_and_barrier
    nc = bass.Bass("TRN2", target_bir_lowering=False, debug=False, num_devices=NCORES)

    h_d = nc.dram_tensor("h", [n, D], F32, kind="ExternalInput")
    w_d = nc.dram_tensor("wqkv", [3, D, D], F32, kind="ExternalInput")
    b_d = nc.dram_tensor("bqkv", [3, D], F32, kind="ExternalInput")
    out_d = nc.dram_tensor("out", [chunk, D], F32, kind="ExternalOutput")

    with tile.TileContext(nc) as tc, ExitStack() as ctx:
        consts = ctx.enter_context(tc.tile_pool(name="consts", bufs=1))
        big = ctx.enter_context(tc.tile_pool(name="big", bufs=1))
        stage = ctx.enter_context(tc.tile_pool(name="stage", bufs=1))
        expp = ctx.enter_context(tc.tile_pool(name="expp", bufs=8))
        treep = ctx.enter_context(tc.tile_pool(name="treep", bufs=10))
        rdenp = ctx.enter_context(tc.tile_pool(name="rdenp", bufs=2))
        outn = ctx.enter_context(tc.tile_pool(name="outn", bufs=2))
        outsp = ctx.enter_context(tc.tile_pool(name="outs", bufs=2))
        outrp = ctx.enter_context(tc.tile_pool(name="outr", bufs=2))
        ps_s = ctx.enter_context(tc.tile_pool(name="ps_s", bufs=3, space="PSUM"))
        ps_o = ctx.enter_context(tc.tile_pool(name="ps_o", bufs=1, space="PSUM"))

        # ---- input DMAs (h first so compute can start early) ----
        w_s = consts.tile([D, 3, D], F32, tag="w_s")
        b_s = consts.tile([D, 3], F32, tag="b_s")
        h_r = h_d.ap().rearrange("(t p) c -> p t c", p=128)
        h_stage = stage.tile([128, M_TILES, 128], F32, tag="h_st")
        nc.sync.dma_start(out=h_stage[:, 0:8, :], in_=h_r[:, 0:8, :])
        nc.sync.dma_start(out=h_stage[:, 8:, :], in_=h_r[:, 8:, :])
        nc.sync.dma_start(out=w_s, in_=w_d.ap().rearrange("w c d -> c w d"))
        nc.sync.dma_start(out=b_s, in_=b_d.ap().rearrange("w d -> d w"))
        h_flat = h_stage

        # ---- constants ----
        wq_r = consts.tile([D, D], F16, tag="wq_r")
        wk_r = consts.tile([D, D], F16, tag="wk_r")
        wv_r = consts.tile([D, D], F16, tag="wv_r")
        nc.vector.tensor_copy(out=wq_r, in_=w_s[:, 0, :])
        nc.vector.tensor_copy(out=wk_r, in_=w_s[:, 1, :])
        nc.vector.tensor_copy(out=wv_r, in_=w_s[:, 2, :])
        bq_s, bk_s, bv_s = b_s[:, 0:1], b_s[:, 1:2], b_s[:, 2:3]
        ones_col = consts.tile([128, 1], F16, tag="ones_col")
        nc.gpsimd.memset(ones_col, 1.0)
        ident = consts.tile([128, 128], F32, tag="ident")
        make_identity(nc, ident)
        ident16 = consts.tile([128, 128], F16, tag="ident16")
        nc.vector.tensor_copy(out=ident16, in_=ident)

        out_r = out_d.ap().rearrange("(t p) d -> p t d", p=128)

        # ---- per-pass operand tiles (double-buffered across passes) ----
        def alloc_gen():
            hT = big.tile([128, n], F16, tag="hT")
            qT = big.tile([128, n], F16, tag="qT")
            kT = big.tile([128, chunk], F16, tag="kT")
            vT = big.tile([128, n], F16, tag="vT")
            vN = big.tile([128, n], F16, tag="vN")
            return {"hT": hT, "qT": qT, "kT": kT, "vT": vT, "vN": vN}

        # ---- prep chunk emitters (h transposes, projections, V-natural) ----
        def c_tgroup(t, g):
            def f():
                t_ps = ps_s.tile([128, GW], F32, tag="s")
                for kk in range(GRP):
                    nc.tensor.transpose(
                        t_ps[:, ts(kk, 128)], h_flat[:, g * GRP + kk, :], ident
                    )
                nc.vector.tensor_copy(out=t["hT"][:, ts(g, GW)], in_=t_ps)

            return f

        def c_proj(t, name, w_r, bias, g):
            def f():
                p_t = ps_s.tile([128, nw], F32, tag="s")
                for j in range(nw // MMW):
                    nc.tensor.matmul(
                        p_t[:, ts(j, MMW)],
                        w_r,
                        t["hT"][:, g * nw + j * MMW : g * nw + (j + 1) * MMW],
                    )
                nc.vector.tensor_scalar_add(
                    out=t[name][:, ts(g, nw)], in0=p_t, scalar1=bias
                )

            return f

        def c_vn(t, g):
            def f():
                t_ps = ps_s.tile([128, GW], F16, tag="s")
                for kk in range(GRP):
                    i = g * GRP + kk
                    nc.tensor.transpose(
                        t_ps[:, ts(kk, 128)], t["vT"][:, ts(i, 128)], ident16
                    )
                nc.vector.tensor_copy(out=t["vN"][:, ts(g, GW)], in_=t_ps)

            return f

        def prep_chunks(t):
            ch = [
                c_tgroup(t, 0),
                c_proj(t, "qT", wq_r, bq_s, 0),
                c_tgroup(t, 1),
                c_proj(t, "kT", wk_r, bk_s, 0),
                c_proj(t, "vT", wv_r, bv_s, 0),
                c_vn(t, 0),
                c_tgroup(t, 2),
                c_proj(t, "qT", wq_r, bq_s, 1),
            ]
            if chunk > nw:
                ch.append(c_proj(t, "kT", wk_r, bk_s, 1))
            ch += [
                c_proj(t, "vT", wv_r, bv_s, 1),
                c_vn(t, 1),
                c_tgroup(t, 3),
                c_proj(t, "qT", wq_r, bq_s, 2),
                c_proj(t, "vT", wv_r, bv_s, 2),
                c_vn(t, 2),
                c_proj(t, "qT", wq_r, bq_s, 3),
                c_proj(t, "vT", wv_r, bv_s, 3),
                c_vn(t, 3),
            ]
            return ch

        state = {"fin": None}

        def make_finalize(nh, o_raw, esum):
            def fin():
                # Transpose the UN-normalized accumulator first (no den
                # dependency, so the PE never waits on the den round trip),
                # and produce the denominator n-partitioned: each
                # esum-chunk x ones-column matmul gives den[n-tile, 1]
                # directly. The normalize then folds into the final
                # per-chunk cast as a per-partition scalar multiply.
                t_ps = ps_s.tile([128, nw], F16, tag="s")
                for kk in range(nw // 128):
                    nc.tensor.transpose(
                        t_ps[:, ts(kk, 128)], o_raw[:, ts(kk, 128)], ident16
                    )
                den_ps = ps_s.tile([128, nw // 128], F32, tag="s")
                for kk in range(nw // 128):
                    nc.tensor.matmul(
                        den_ps[:, kk : kk + 1],
                        esum[:, ts(kk, 128)],
                        ones_col,
                    )
                rden_n = rdenp.tile([128, nw // 128], F32, tag="rden")
                nc.vector.reciprocal(out=rden_n, in_=den_ps)
                t_v = t_ps.rearrange("p (t d) -> p t d", d=128)
                o_s = outsp.tile([128, nw // 128, 128], F32, tag="o_s")
                for kk in range(nw // 128):
                    nc.vector.tensor_scalar_mul(
                        out=o_s[:, kk, :],
                        in0=t_v[:, kk, :],
                        scalar1=rden_n[:, kk : kk + 1],
                    )
                nc.sync.dma_start(
                    out=out_r[:, nh * (nw // 128) : (nh + 1) * (nw // 128), :],
                    in_=o_s,
                )

            return fin

        def main_pass(t, prep_list, positions=None):
            if positions is None:
                # spread next-pass prep chunks over the 64 iters (ping-pong)
                positions = {}
                if prep_list:
                    npc = len(prep_list)
                    for i, c in enumerate(prep_list):
                        kpos = 2 + (i * 58) // npc
                        positions.setdefault(kpos, []).append(c)

            def emit_scores(nh, mi):
                s_t = ps_s.tile([128, nw], F32, tag="s")
                for j in range(nw // MMW):
                    nc.tensor.matmul(
                        s_t[:, ts(j, MMW)],
                        t["qT"][:, ts(mi, 128)],
                        t["kT"][:, nh * nw + j * MMW : nh * nw + (j + 1) * MMW],
                    )
                return s_t

            s_next = emit_scores(0, 0)
            o_t = None
            tree = []
            for k in range(TOT):
                nh, mi = divmod(k, M_TILES)
                if mi == 0:
                    o_t = ps_o.tile([128, nw], F32, tag="o")
                    tree = [[] for _ in range(6)]
                s_t = s_next
                e_t = expp.tile([128, nw], F16, tag="e")
                nc.scalar.activation(
                    out=e_t,
                    in_=s_t,
                    func=mybir.ActivationFunctionType.Exp,
                    scale=SCALE,
                )
                if k + 1 < TOT:
                    nh2, mi2 = divmod(k + 1, M_TILES)
                    s_next = emit_scores(nh2, mi2)
                first, last = mi == 0, mi == M_TILES - 1
                for j in range(nw // MMW):
                    nc.tensor.matmul(
                        o_t[:, ts(j, MMW)],
                        t["vN"][:, ts(mi, 128)],
                        e_t[:, ts(j, MMW)],
                        start=first,
                        stop=last,
                        skip_group_check=True,
                    )
                if k in positions:
                    for c in positions[k]:
                        c()
                if state["fin"] is not None and mi == 6:
                    state["fin"]()
                    state["fin"] = None
                o_raw = None
                if last:
                    # drain the PV accumulator ahead of the tree adds so the
                    # single PSUM o-slot frees ASAP for the next window
                    o_raw = outrp.tile([128, nw], F16, tag="o_raw")
                    nc.vector.tensor_copy(out=o_raw, in_=o_t)
                # denominator tree: pairwise-add e tiles on DVE (fp16, 2x)
                node, lvl = e_t, 0
                while tree[lvl]:
                    prev = tree[lvl].pop()
                    nxt = treep.tile([128, nw], F16, tag="t")
                    nc.vector.tensor_add(out=nxt, in0=prev, in1=node)
                    node, lvl = nxt, lvl + 1
                tree[lvl].append(node)
                if last:
                    esum = tree[5][0] if tree[5] else tree[4][0]
                    state["fin"] = make_finalize(nh, o_raw, esum)

        # ---- emission ----
        def self_prep_body(t):
            # per-pass prep: minimal prologue, the rest interleaved into
            # window 0 under its data deadlines (qT w_g before mi=8g, etc.)
            c_tgroup(t, 0)()
            c_proj(t, "kT", wk_r, bk_s, 0)()
            c_proj(t, "qT", wq_r, bq_s, 0)()
            c_proj(t, "vT", wv_r, bv_s, 0)()
            c_vn(t, 0)()
            positions = {
                1: [c_tgroup(t, 1)],
                2: [c_proj(t, "qT", wq_r, bq_s, 1)],
                3: [c_tgroup(t, 2)],
                4: [c_proj(t, "vT", wv_r, bv_s, 1)],
                5: [c_tgroup(t, 3)],
                6: [c_vn(t, 1)],
                10: [c_proj(t, "qT", wq_r, bq_s, 2)],
                12: [c_proj(t, "vT", wv_r, bv_s, 2)],
                14: [c_vn(t, 2)],
                17: [c_proj(t, "kT", wk_r, bk_s, 1)] if chunk > nw else [],
                18: [c_proj(t, "qT", wq_r, bq_s, 3)],
                20: [c_proj(t, "vT", wv_r, bv_s, 3)],
                22: [c_vn(t, 3)],
            }
            main_pass(t, None, positions=positions)

        if repeat <= 1:
            t_a = alloc_gen()
            for c in prep_chunks(t_a):
                c()
            main_pass(t_a, [])
            state["fin"]()
            state["fin"] = None
        elif pingpong:
            # NOTE: cross-pass ping-pong was measured slower (out-of-loop
            # tiles carry whole-tile wrap dependencies) and the in-loop
            # variant deadlocks the tile scheduler (read-before-write across
            # the back edge). Kept only as a guarded experiment flag.
            raise NotImplementedError("pingpong mode disabled")
        else:
            with tc.For_i(0, repeat, 1):
                # allocate per-pass operand tiles INSIDE the loop body: the
                # scheduler then tracks slot reuse at pool granularity (as in
                # the fastest measured configuration) instead of carrying
                # whole-tile wrap dependencies
                self_prep_body(alloc_gen())
                state["fin"]()
                state["fin"] = None

    if split_waits:
        _split_excess_waits(nc)
    return nc


_NC_CACHE = None
_LAST_RESULTS = None
TRACE = False
REPEAT = 1
FLAGS = {}


def kernel(h_a, Wq, bq, Wk, bk, Wv, bv):
    global _NC_CACHE, _LAST_RESULTS
    h_a = np.ascontiguousarray(h_a, dtype=np.float32)
    if _NC_CACHE is None:
        _NC_CACHE = build_nc(repeat=REPEAT, **FLAGS)
    nc = _NC_CACHE

    consts = {
        "wqkv": np.ascontiguousarray(np.stack([Wq, Wk, Wv]), np.float32),
        "bqkv": np.ascontiguousarray(np.stack([bq, bk, bv]), np.float32),
    }
    in_maps = []
    for core in range(NCORES):
        b, half = divmod(core, 2)
        n0 = half * CHUNK
        # chunk rows first, the rest after (order of the tail is irrelevant)
        perm = np.concatenate(
            [h_a[b, n0 : n0 + CHUNK], h_a[b, : n0], h_a[b, n0 + CHUNK :]], axis=0
        )
        in_maps.append({"h": np.ascontiguousarray(perm), **consts})

    res = run_bass_kernel_spmd(
        nc, in_maps, core_ids=list(range(NCORES)), trace=TRACE
    )
    _LAST_RESULTS = res

    out = np.empty((B, N, D), np.float32)
    for core in range(NCORES):
        b, half = divmod(core, 2)
        n0 = half * CHUNK
        out[b, n0 : n0 + CHUNK] = res.results[core]["out"]
    return out


# revision 32
# speedup vs baseline: 1.2956x; 1.2956x over previous
"""Trainium2 Bass kernel for nn_AttentionLayer (B=4, N=4096, D=128).

Computation (per reference):
    Q = h @ Wq + bq ; K = h @ Wk + bk ; V = h @ Wv + bv          [B, N, 128]
    scores = einsum("bnd,bmd->bnm", K, Q) / sqrt(128)            [B, N, N]
    attn = softmax(scores, axis=-1)
    out = einsum("bnm,bmd->bnd", attn, V)                        [B, N, N->128]

Sharding: 8 cores = 4 batches x 2 chunks of 2048 K-rows (output rows).
Fully data-parallel SPMD - no collectives. Each core receives its batch's
h rows PERMUTED so that its own K-chunk rows come first: softmax/PV reduce
over the m (Q/V) index, which is order-independent, so the permutation only
fixes which rows the core treats as its K chunk (the first 2048).

Design (v4):
  - ACT exp of the full [2048, 4096] score block (64 x [128,1024] tiles)
    is the hard per-core floor (~66us at 1.2GHz x 128 lanes); everything
    else is organized to keep ACT saturated.
  - All matmul operands fp16 (rel err vs fp32 reference ~3e-4, gate 2e-2).
  - Main loop per (window nh, q-tile mi): PE scores -> ACT exp -> PE PV
    accumulate. The softmax denominator is NOT a third PE pass over the
    score block: DVE pairwise-adds the 32 fp16 e-tiles of each window
    (31 adds at the 2x packed mode) and a single ones[128,128] matmul on
    the reduced esum gives the column sums broadcast over partitions
    (2 x 1024 cols of PE instead of 64 x 1024).
  - PE transposes (h^T, V-natural, output) with a bf16/fp16 identity;
    DMA-XBAR transposes and gpsimd bulk ops measured far slower on HW.
  - Window finalize (den matmul, recip, normalize, transpose, store) is
    deferred into the next window so PE/ACT never wait on the den
    round-trip; the PV accumulator is drained to SBUF at window end so
    the single PSUM o-slot frees immediately.
  - Ping-pong software pipeline across repeat iterations: the For_i body
    holds TWO passes; each pass interleaves the NEXT pass's h-transposes
    and Q/K/V projections (into the other buffer of each double-buffered
    operand tile) into its own slack, so the steady-state per-pass cost
    has no projection prologue.
"""

import math
from contextlib import ExitStack

import numpy as np

import concourse.bass as bass
import concourse.mybir as mybir
import concourse.tile as tile
from concourse.bass_utils import run_bass_kernel_spmd
from concourse.masks import make_identity
from concourse.tile import ScopedClock

F32 = mybir.dt.float32
F16 = mybir.dt.float16

B, N, D = 4, 4096, 128
NCORES = 8
CHUNK = N * B // NCORES  # 2048 output rows per core
NW = 1024  # n processed per PSUM-resident accumulation group
SCALE = 1.0 / math.sqrt(D)


def _patched_drain_and_barrier(self, tick_clock, wait_clock):
    # This walrus build rejects multiple sync waits on the Drain CTRL
    # instruction. Carry the waits on preceding SP nops (same engine =>
    # program order) and leave the drain nearly bare.
    nc = self.nc
    carrier = nc.sync.nop(nofuse=True, hint="drain_waits")
    wait_clock.add_sem_waits(carrier.ins, ScopedClock({None: tick_clock.global_clock}))
    si = carrier.ins.sync_info
    waits = list(si.on_wait) if si is not None else []
    if len(waits) > 1:
        by_name = {}
        for h in self.sems.allocated().values():
            by_name[getattr(h, "name", None) or str(h)] = h
        si.on_wait = [waits[0]]
        for w in waits[1:]:
            n = nc.sync.nop(nofuse=True, hint="drain_waits2")
            n.wait_op(by_name[w.ant_name], w.wait_value, "sem-ge")
    nc.sync.drain()
    nc.all_engine_barrier()
    assert self.sems is not None
    popped = nc._tile_sem_poison_stack.pop()
    assert popped is self._sem_poison
    nc.clear_and_free_semaphores(list(self.sems.allocated().values()))
    nc.all_engine_barrier()


def ts(i, sz):
    return slice(i * sz, (i + 1) * sz)


def _split_excess_waits(nc, maxw=1):
    # This walrus build allows at most ~1 sync wait per lowered instruction.
    # Hoist excess waits onto preceding same-engine NoOps.
    cnt = 0
    for f in nc.m.functions:
        for bb in f.blocks:
            out = []
            for inst in bb.instructions:
                si = inst.sync_info
                waits = list(si.on_wait) if si is not None else []
                if len(waits) > maxw:
                    for w in waits[: len(waits) - maxw]:
                        nop = mybir.InstNoOp(
                            name=f"{inst.name}-hw{cnt}",
                            engine=inst.engine,
                            ins=[],
                            outs=[],
                            sync_info=mybir.SyncInfo(on_wait=[w], on_update=[]),
                        )
                        out.append(nop)
                        cnt += 1
                    si.on_wait = waits[len(waits) - maxw :]
                out.append(inst)
            bb.instructions = out
    return cnt


def build_nc(n=N, chunk=CHUNK, nw=NW, split_waits=True, repeat=1,
             pingpong=False, **_flags):
    M_TILES = n // 128  # 32
    NH = chunk // nw  # 2
    MMW = min(512, nw)  # matmul moving width (PSUM bank cap for fp32)
    GRP = 8  # tiles per prep group (1024 cols)
    N_GRP = M_TILES // GRP  # 4
    GW = GRP * 128
    TOT = NH * M_TILES
    tile.TileContext._drain_and_barrier = _patched_drain_and_barrier
    nc = bass.Bass("TRN2", target_bir_lowering=False, debug=False, num_devices=NCORES)

    h_d = nc.dram_tensor("h", [n, D], F32, kind="ExternalInput")
    w_d = nc.dram_tensor("wqkv", [3, D, D], F32, kind="ExternalInput")
    b_d = nc.dram_tensor("bqkv", [3, D], F32, kind="ExternalInput")
    out_d = nc.dram_tensor("out", [chunk, D], F32, kind="ExternalOutput")

    with tile.TileContext(nc) as tc, ExitStack() as ctx:
        consts = ctx.enter_context(tc.tile_pool(name="consts", bufs=1))
        big = ctx.enter_context(tc.tile_pool(name="big", bufs=1))
        stage = ctx.enter_context(tc.tile_pool(name="stage", bufs=1))
        expp = ctx.enter_context(tc.tile_pool(name="expp", bufs=8))
        treep = ctx.enter_context(tc.tile_pool(name="treep", bufs=10))
        rdenp = ctx.enter_context(tc.tile_pool(name="rdenp", bufs=2))
        outn = ctx.enter_context(tc.tile_pool(name="outn", bufs=2))
        outsp = ctx.enter_context(tc.tile_pool(name="outs", bufs=2))
        outrp = ctx.enter_context(tc.tile_pool(name="outr", bufs=2))
        ps_s = ctx.enter_context(tc.tile_pool(name="ps_s", bufs=3, space="PSUM"))
        ps_o = ctx.enter_context(tc.tile_pool(name="ps_o", bufs=1, space="PSUM"))

        # ---- input DMAs (h first so compute can start early) ----
        w_s = consts.tile([D, 3, D], F32, tag="w_s")
        b_s = consts.tile([D, 3], F32, tag="b_s")
        h_r = h_d.ap().rearrange("(t p) c -> p t c", p=128)
        h_stage = stage.tile([128, M_TILES, 128], F32, tag="h_st")
        nc.sync.dma_start(out=h_stage[:, 0:8, :], in_=h_r[:, 0:8, :])
        nc.sync.dma_start(out=h_stage[:, 8:, :], in_=h_r[:, 8:, :])
        nc.sync.dma_start(out=w_s, in_=w_d.ap().rearrange("w c d -> c w d"))
        nc.sync.dma_start(out=b_s, in_=b_d.ap().rearrange("w d -> d w"))
        h_flat = h_stage

        # ---- constants ----
        wq_r = consts.tile([D, D], F16, tag="wq_r")
        wk_r = consts.tile([D, D], F16, tag="wk_r")
        wv_r = consts.tile([D, D], F16, tag="wv_r")
        nc.vector.tensor_copy(out=wq_r, in_=w_s[:, 0, :])
        nc.vector.tensor_copy(out=wk_r, in_=w_s[:, 1, :])
        nc.vector.tensor_copy(out=wv_r, in_=w_s[:, 2, :])
        bq_s, bk_s, bv_s = b_s[:, 0:1], b_s[:, 1:2], b_s[:, 2:3]
        ones_col = consts.tile([128, 1], F16, tag="ones_col")
        nc.gpsimd.memset(ones_col, 1.0)
        ident = consts.tile([128, 128], F32, tag="ident")
        make_identity(nc, ident)
        ident16 = consts.tile([128, 128], F16, tag="ident16")
        nc.vector.tensor_copy(out=ident16, in_=ident)

        out_r = out_d.ap().rearrange("(t p) d -> p t d", p=128)

        # ---- per-pass operand tiles (double-buffered across passes) ----
        def alloc_gen():
            hT = big.tile([128, n], F16, tag="hT")
            qT = big.tile([128, n], F16, tag="qT")
            kT = big.tile([128, chunk], F16, tag="kT")
            vT = big.tile([128, n], F16, tag="vT")
            vN = big.tile([128, n], F16, tag="vN")
            return {"hT": hT, "qT": qT, "kT": kT, "vT": vT, "vN": vN}

        # ---- prep chunk emitters (h transposes, projections, V-natural) ----
        def c_tgroup(t, g):
            def f():
                t_ps = ps_s.tile([128, GW], F32, tag="s")
                for kk in range(GRP):
                    nc.tensor.transpose(
                        t_ps[:, ts(kk, 128)], h_flat[:, g * GRP + kk, :], ident
                    )
                nc.vector.tensor_copy(out=t["hT"][:, ts(g, GW)], in_=t_ps)

            return f

        def c_proj(t, name, w_r, bias, g):
            def f():
                p_t = ps_s.tile([128, nw], F32, tag="s")
                for j in range(nw // MMW):
                    nc.tensor.matmul(
                        p_t[:, ts(j, MMW)],
                        w_r,
                        t["hT"][:, g * nw + j * MMW : g * nw + (j + 1) * MMW],
                    )
                nc.vector.tensor_scalar_add(
                    out=t[name][:, ts(g, nw)], in0=p_t, scalar1=bias
                )

            return f

        def c_vn(t, g):
            def f():
                t_ps = ps_s.tile([128, GW], F16, tag="s")
                for kk in range(GRP):
                    i = g * GRP + kk
                    nc.tensor.transpose(
                        t_ps[:, ts(kk, 128)], t["vT"][:, ts(i, 128)], ident16
                    )
                nc.vector.tensor_copy(out=t["vN"][:, ts(g, GW)], in_=t_ps)

            return f

        def prep_chunks(t):
            ch = [
                c_tgroup(t, 0),
                c_proj(t, "qT", wq_r, bq_s, 0),
                c_tgroup(t, 1),
                c_proj(t, "kT", wk_r, bk_s, 0),
                c_proj(t, "vT", wv_r, bv_s, 0),
                c_vn(t, 0),
                c_tgroup(t, 2),
                c_proj(t, "qT", wq_r, bq_s, 1),
            ]
            if chunk > nw:
                ch.append(c_proj(t, "kT", wk_r, bk_s, 1))
            ch += [
                c_proj(t, "vT", wv_r, bv_s, 1),
                c_vn(t, 1),
                c_tgroup(t, 3),
                c_proj(t, "qT", wq_r, bq_s, 2),
                c_proj(t, "vT", wv_r, bv_s, 2),
                c_vn(t, 2),
                c_proj(t, "qT", wq_r, bq_s, 3),
                c_proj(t, "vT", wv_r, bv_s, 3),
                c_vn(t, 3),
            ]
            return ch

        state = {"fin": None}

        def make_finalize(nh, o_raw, esum):
            def fin():
                # Transpose the UN-normalized accumulator first (no den
                # dependency, so the PE never waits on the den round trip),
                # and produce the denominator n-partitioned: each
                # esum-chunk x ones-column matmul gives den[n-tile, 1]
                # directly. The normalize then folds into the final
                # per-chunk cast as a per-partition scalar multiply.
                t_ps = ps_s.tile([128, nw], F16, tag="s")
                for kk in range(nw // 128):
                    nc.tensor.transpose(
                        t_ps[:, ts(kk, 128)], o_raw[:, ts(kk, 128)], ident16
                    )
                den_ps = ps_s.tile([128, nw // 128], F32, tag="s")
                for kk in range(nw // 128):
                    nc.tensor.matmul(
                        den_ps[:, kk : kk + 1],
                        esum[:, ts(kk, 128)],
                        ones_col,
                    )
                rden_n = rdenp.tile([128, nw // 128], F32, tag="rden")
                nc.vector.reciprocal(out=rden_n, in_=den_ps)
                t_v = t_ps.rearrange("p (t d) -> p t d", d=128)
                o_s = outsp.tile([128, nw // 128, 128], F32, tag="o_s")
                for kk in range(nw // 128):
                    nc.vector.tensor_scalar_mul(
                        out=o_s[:, kk, :],
                        in0=t_v[:, kk, :],
                        scalar1=rden_n[:, kk : kk + 1],
                    )
                nc.sync.dma_start(
                    out=out_r[:, nh * (nw // 128) : (nh + 1) * (nw // 128), :],
                    in_=o_s,
                )

            return fin

        def main_pass(t, prep_list, positions=None, prepv=None):
            if positions is None:
                # spread next-pass prep chunks over the 64 iters (ping-pong)
                positions = {}
                if prep_list:
                    npc = len(prep_list)
                    for i, c in enumerate(prep_list):
                        kpos = 2 + (i * 58) // npc
                        positions.setdefault(kpos, []).append(c)

            def emit_scores(nh, mi):
                s_t = ps_s.tile([128, nw], F32, tag="s")
                for j in range(nw // MMW):
                    nc.tensor.matmul(
                        s_t[:, ts(j, MMW)],
                        t["qT"][:, ts(mi, 128)],
                        t["kT"][:, nh * nw + j * MMW : nh * nw + (j + 1) * MMW],
                    )
                return s_t

            s_next = emit_scores(0, 0)
            o_t = None
            tree = []
            for k in range(TOT):
                nh, mi = divmod(k, M_TILES)
                if mi == 0:
                    o_t = ps_o.tile([128, nw], F32, tag="o")
                    tree = [[] for _ in range(6)]
                s_t = s_next
                e_t = expp.tile([128, nw], F16, tag="e")
                nc.scalar.activation(
                    out=e_t,
                    in_=s_t,
                    func=mybir.ActivationFunctionType.Exp,
                    scale=SCALE,
                )
                if k + 1 < TOT:
                    nh2, mi2 = divmod(k + 1, M_TILES)
                    s_next = emit_scores(nh2, mi2)
                if prepv and k in prepv:
                    # V-path prep: must precede this iteration's PV in PE
                    # program order, but nothing ACT needs waits on it
                    for c in prepv[k]:
                        c()
                first, last = mi == 0, mi == M_TILES - 1
                for j in range(nw // MMW):
                    nc.tensor.matmul(
                        o_t[:, ts(j, MMW)],
                        t["vN"][:, ts(mi, 128)],
                        e_t[:, ts(j, MMW)],
                        start=first,
                        stop=last,
                        skip_group_check=True,
                    )
                if k in positions:
                    for c in positions[k]:
                        c()
                if state["fin"] is not None and mi == 6:
                    state["fin"]()
                    state["fin"] = None
                o_raw = None
                if last:
                    # drain the PV accumulator ahead of the tree adds so the
                    # single PSUM o-slot frees ASAP for the next window
                    o_raw = outrp.tile([128, nw], F16, tag="o_raw")
                    nc.vector.tensor_copy(out=o_raw, in_=o_t)
                # denominator tree: pairwise-add e tiles on DVE (fp16, 2x)
                node, lvl = e_t, 0
                while tree[lvl]:
                    prev = tree[lvl].pop()
                    nxt = treep.tile([128, nw], F16, tag="t")
                    nc.vector.tensor_add(out=nxt, in0=prev, in1=node)
                    node, lvl = nxt, lvl + 1
                tree[lvl].append(node)
                if last:
                    esum = tree[5][0] if tree[5] else tree[4][0]
                    state["fin"] = make_finalize(nh, o_raw, esum)

        # ---- emission ----
        def self_prep_body(t):
            # per-pass prep: ONLY what the first scores needs, with the
            # kT/qT drains split in halves and interleaved on DVE so
            # scores(0,0) (and hence the first exp) issues ~2.5us sooner;
            # the V path moves to just before the first PV via prepv.
            t_ps = ps_s.tile([128, GW], F32, tag="s")
            for kk in range(GRP):
                nc.tensor.transpose(
                    t_ps[:, ts(kk, 128)], h_flat[:, kk, :], ident
                )
            nc.vector.tensor_copy(out=t["hT"][:, 0:GW], in_=t_ps)
            pk = ps_s.tile([128, nw], F32, tag="s")
            pq = ps_s.tile([128, nw], F32, tag="s")
            for j in range(nw // MMW):
                nc.tensor.matmul(
                    pk[:, ts(j, MMW)], wk_r, t["hT"][:, ts(j, MMW)]
                )
            for j in range(nw // MMW):
                nc.tensor.matmul(
                    pq[:, ts(j, MMW)], wq_r, t["hT"][:, ts(j, MMW)]
                )
            for j in range(nw // MMW):
                nc.vector.tensor_scalar_add(
                    out=t["kT"][:, ts(j, MMW)], in0=pk[:, ts(j, MMW)],
                    scalar1=bk_s,
                )
                nc.vector.tensor_scalar_add(
                    out=t["qT"][:, ts(j, MMW)], in0=pq[:, ts(j, MMW)],
                    scalar1=bq_s,
                )
            prepv = {0: [c_proj(t, "vT", wv_r, bv_s, 0), c_vn(t, 0)]}
            positions = {
                1: [c_tgroup(t, 1)],
                2: [c_proj(t, "qT", wq_r, bq_s, 1)],
                3: [c_tgroup(t, 2)],
                4: [c_proj(t, "vT", wv_r, bv_s, 1)],
                5: [c_tgroup(t, 3)],
                6: [c_vn(t, 1)],
                10: [c_proj(t, "qT", wq_r, bq_s, 2)],
                12: [c_proj(t, "vT", wv_r, bv_s, 2)],
                14: [c_vn(t, 2)],
                17: [c_proj(t, "kT", wk_r, bk_s, 1)] if chunk > nw else [],
                18: [c_proj(t, "qT", wq_r, bq_s, 3)],
                20: [c_proj(t, "vT", wv_r, bv_s, 3)],
                22: [c_vn(t, 3)],
            }
            main_pass(t, None, positions=positions, prepv=prepv)

        if repeat <= 1:
            t_a = alloc_gen()
            for c in prep_chunks(t_a):
                c()
            main_pass(t_a, [])
            state["fin"]()
            state["fin"] = None
        elif pingpong:
            # NOTE: cross-pass ping-pong was measured slower (out-of-loop
            # tiles carry whole-tile wrap dependencies) and the in-loop
            # variant deadlocks the tile scheduler (read-before-write across
            # the back edge). Kept only as a guarded experiment flag.
            raise NotImplementedError("pingpong mode disabled")
        else:
            with tc.For_i(0, repeat, 1):
                # allocate per-pass operand tiles INSIDE the loop body: the
                # scheduler then tracks slot reuse at pool granularity (as in
                # the fastest measured configuration) instead of carrying
                # whole-tile wrap dependencies
                self_prep_body(alloc_gen())
                state["fin"]()
                state["fin"] = None

    if split_waits:
        _split_excess_waits(nc)
    return nc


_NC_CACHE = None
_LAST_RESULTS = None
TRACE = False
REPEAT = 1
FLAGS = {}


def kernel(h_a, Wq, bq, Wk, bk, Wv, bv):
    global _NC_CACHE, _LAST_RESULTS
    h_a = np.ascontiguousarray(h_a, dtype=np.float32)
    if _NC_CACHE is None:
        _NC_CACHE = build_nc(repeat=REPEAT, **FLAGS)
    nc = _NC_CACHE

    consts = {
        "wqkv": np.ascontiguousarray(np.stack([Wq, Wk, Wv]), np.float32),
        "bqkv": np.ascontiguousarray(np.stack([bq, bk, bv]), np.float32),
    }
    in_maps = []
    for core in range(NCORES):
        b, half = divmod(core, 2)
        n0 = half * CHUNK
        # chunk rows first, the rest after (order of the tail is irrelevant)
        perm = np.concatenate(
            [h_a[b, n0 : n0 + CHUNK], h_a[b, : n0], h_a[b, n0 + CHUNK :]], axis=0
        )
        in_maps.append({"h": np.ascontiguousarray(perm), **consts})

    res = run_bass_kernel_spmd(
        nc, in_maps, core_ids=list(range(NCORES)), trace=TRACE
    )
    _LAST_RESULTS = res

    out = np.empty((B, N, D), np.float32)
    for core in range(NCORES):
        b, half = divmod(core, 2)
        n0 = half * CHUNK
        out[b, n0 : n0 + CHUNK] = res.results[core]["out"]
    return out
